# revision 25
# baseline (speedup 1.0000x reference)
"""Trainium2 Bass kernel for nn_LocalFeatureGuided.

Pipeline per image (C=128 on partitions, spatial on free dim):
  BN(eval)+GELU (ACT, fused affine) -> even/odd column split buffers
  depthwise 7x7 s2 conv: 49 taps split by output-column ranges across
    PE (diag-matmul, PSUM accum), DVE (scalar_tensor_tensor FMA), GPSIMD
  tokens: t0=guide, t1..4 = strided views of x (no copies)
  q0 = WqT.T@t0 ; k_m = WkT.T@t_m consumed from PSUM by
    tensor_tensor_reduce dots -> s_m = <q0,k_m>
  softmax over 5 logits per (b,c); v & proj fused:
    out = sum_m (Wv^T diag(a_m) Pw^T)^T @ t_m  (5 accumulating matmuls)
Sharding: data-parallel over batch, 2 images per core, 8 cores.
"""

import os
import numpy as np
from contextlib import ExitStack

import concourse.bass as bass
import concourse.tile as tile
from concourse import bacc, mybir
from concourse import bass_utils
from concourse import tile_utils

alu = mybir.AluOpType
actf = mybir.ActivationFunctionType
F32 = mybir.dt.float32
F32R = mybir.dt.float32r
BF16 = mybir.dt.bfloat16

B, C, H, W = 16, 128, 128, 128
H2, W2 = H // 2, W // 2
L = H2 * W2            # 4096
NCORES = 8
BPC = B // NCORES      # 2 images per core
EPS = 1e-5
INV_SQRT_C = 1.0 / np.sqrt(128.0)

KSTAGE = int(os.environ.get("KSTAGE", "9"))
KREPEAT = int(os.environ.get("KREPEAT", "1"))  # timing: repeat image loop

# ---- tuning knobs ----
PE_H2 = 36             # conv: h2 rows 0..PE_H2 on PE
# DVE handles rows [PE_H2, 64) in bf16 via shadow buffers V0/V1.
EO_R = 2 * PE_H2 + 6   # eo rows kept in f32r (PE reads rows < EO_R)
SH_R0 = 2 * PE_H2      # first eo-space row in the bf16 shadow
NSH = 134 - SH_R0      # shadow rows per parity (incl 3 bottom pad rows)
MM_F32R = True         # attention matmuls in float32r (1 cyc/col vs 4)
CONV_F32R = True       # conv diag matmuls in float32r
TAP_BLOCK = 17         # diag weight tiles alive at once
SBUF_CAP = 204 * 1024  # cayman has 208 KiB usable per partition

KH_LO = [2, 1, 1, 0, 0, 0, 0]
KH_HI = [64, 64, 64, 64, 64, 63, 63]


def _mmdt(ap, enable):
    return ap.bitcast(F32R) if enable else ap


_rnd = _mmdt  # producers of matmul operands must round to f32r on write


def tap_geometry(kh, kw):
    """Returns (parity, u, h2 range, w2 range) for tap (kh, kw)."""
    e = kw - 3
    if e % 2 == 0:
        par, u = 0, e // 2          # even: reads E[r, w2+u], u in -1..1
    else:
        par, u = 1, (e - 1) // 2    # odd: reads O[r, w2+u], u in -2..1
    wlo, whi = max(0, -u), min(64, 64 - u)
    return par, u, KH_LO[kh], KH_HI[kh], wlo, whi


def build(nc):
    x_d = nc.dram_tensor("x", (BPC, C, H, W), F32, kind="ExternalInput").ap()
    bng_d = nc.dram_tensor("bn_gamma", (C, 1), F32, kind="ExternalInput").ap()
    bnb_d = nc.dram_tensor("bn_beta", (C, 1), F32, kind="ExternalInput").ap()
    bnm_d = nc.dram_tensor("bn_mean", (C, 1), F32, kind="ExternalInput").ap()
    bnv_d = nc.dram_tensor("bn_var", (C, 1), F32, kind="ExternalInput").ap()
    dww_d = nc.dram_tensor("dw_w", (C, 49), F32, kind="ExternalInput").ap()
    dwb_d = nc.dram_tensor("dw_b", (C, 1), F32, kind="ExternalInput").ap()
    qkvw_d = nc.dram_tensor("qkv_w", (3 * C, C), F32, kind="ExternalInput").ap()
    qkvb_d = nc.dram_tensor("qkv_b", (3 * C, 1), F32, kind="ExternalInput").ap()
    pw_d = nc.dram_tensor("proj_w", (C, C), F32, kind="ExternalInput").ap()
    pb_d = nc.dram_tensor("proj_b", (C, 1), F32, kind="ExternalInput").ap()
    out_d = nc.dram_tensor("out", (BPC, C, H2, W2), F32, kind="ExternalOutput").ap()

    with tile.TileContext(nc) as tc, ExitStack() as ctx:
        tp = lambda name, bufs, **kw: ctx.enter_context(
            tc.tile_pool(name=name, bufs=bufs, **kw))

        wpool = tp("weights", 1)       # persistent small weights
        diagp = tp("diag", TAP_BLOCK)  # conv diag weight tiles
        xp = tp("x", 1)
        gp = tp("gelu", 1)
        t0p = tp("t0", 1)
        q0p = tp("q0", 1)
        accp = tp("acc", 1)
        vshp = tp("vsh", 1)
        outp = tp("outc", 2)
        vecp = tp("vec", 24)
        emp = tp("em", 2)
        scrp = tp("scr", 1)
        xsp = tp("xstg", 1)
        vtp = tp("vt", 5)
        ksbp = tp("ksb", 3)
        pp512 = tp("pp512", 2, space="PSUM")
        ppk = tp("ppk", 2, space="PSUM")   # [128,1024] = 2 banks each

        # ---------- phase 0: weights & per-channel vectors ----------
        def vec_load(src_ap):
            t = vecp.tile([C, 1], F32, tag="v")
            nc.sync.dma_start(t[:], src_ap)
            return t

        gam = vec_load(bng_d)
        bet = vec_load(bnb_d)
        mea = vec_load(bnm_d)
        var = vec_load(bnv_d)
        dwb = vec_load(dwb_d)
        bq = vec_load(qkvb_d[0:C])
        bk = vec_load(qkvb_d[C:2 * C])
        bv = vec_load(qkvb_d[2 * C:3 * C])
        pb = vec_load(pb_d)

        dww = wpool.tile([C, 49], F32)
        nc.sync.dma_start(dww[:], dww_d)

        wq_raw = wpool.tile([C, C], F32)
        nc.sync.dma_start(wq_raw[:], qkvw_d[0:C])
        wk_raw = wpool.tile([C, C], F32)
        nc.sync.dma_start(wk_raw[:], qkvw_d[C:2 * C])
        wv_raw = wpool.tile([C, C], F32)          # used directly as lhsT
        nc.sync.dma_start(wv_raw[:], qkvw_d[2 * C:3 * C])
        pw_raw = wpool.tile([C, C], F32)
        nc.sync.dma_start(pw_raw[:], pw_d)

        # identity for transposes / diag builds
        ones = wpool.tile([C, C], F32)
        nc.gpsimd.memset(ones[:], 1.0)
        ident = wpool.tile([C, C], F32)
        nc.gpsimd.affine_select(ident[:], ones[:], [[-1, C]], alu.is_equal,
                                0.0, base=0, channel_multiplier=1)

        def pe_transpose(src, rnd_out=False):
            ps = pp512.tile([C, 512], F32, tag="w")
            nc.tensor.matmul(ps[:, 0:C], src[:], ident[:], start=True,
                             stop=True, is_transpose=True)
            dst = wpool.tile([C, C], F32, tag=f"T{src.name}")
            nc.scalar.copy(_rnd(dst[:], rnd_out), ps[:, 0:C])
            return dst

        wqT = pe_transpose(wq_raw, rnd_out=MM_F32R)   # [j, cq]
        wkT = pe_transpose(wk_raw, rnd_out=MM_F32R)   # [j, ck]
        pwT = pe_transpose(pw_raw)                    # [o, d]
        if MM_F32R:
            # wv_raw is a matmul lhsT; DMA can't round, so stage a rounded copy
            wv_use = wpool.tile([C, C], F32, tag="wvr")
            nc.scalar.copy(wv_use[:].bitcast(F32R), wv_raw[:])
        else:
            wv_use = wv_raw

        # BN affine: scale = gamma*rsqrt(var+eps); bias = beta - mean*scale
        epst = vecp.tile([C, 1], F32, tag="v")
        nc.gpsimd.memset(epst[:], EPS)
        std = vecp.tile([C, 1], F32, tag="v")
        nc.scalar.activation(std[:], var[:], actf.Sqrt, bias=epst[:, 0:1],
                             scale=1.0)
        istd = vecp.tile([C, 1], F32, tag="v")
        nc.vector.reciprocal(istd[:], std[:])
        bns = vecp.tile([C, 1], F32, tag="v")
        nc.vector.tensor_tensor(bns[:], gam[:], istd[:], alu.mult)
        bnsn = vecp.tile([C, 1], F32, tag="v")
        nc.vector.tensor_scalar_mul(bnsn[:], bns[:], -1.0)
        bnb = vecp.tile([C, 1], F32, tag="v")
        nc.vector.scalar_tensor_tensor(bnb[:], mea[:], bnsn[:], bet[:],
                                       alu.mult, alu.add)

        # cb = Pw @ bv + pb   (softmax weights sum to 1 -> v-bias folds out)
        cps = pp512.tile([C, 512], F32, tag="w")
        nc.tensor.matmul(cps[:, 0:1], pwT[:], bv[:], start=True, stop=True)
        cb = vecp.tile([C, 1], F32, tag="v")
        nc.scalar.activation(cb[:], cps[:, 0:1], actf.Identity,
                             bias=pb[:, 0:1])

        taps = [(kh, kw) for kh in range(7) for kw in range(7)]

        # ---------- per image ----------
        for img in [i % BPC for i in range(BPC * KREPEAT)]:
            x_t = xp.tile([C, H * W], F32)
            xi = x_d[img].rearrange("c h w -> c (h w)")
            for s in range(16):  # parallel DMA queues
                sl = slice(s * 1024, (s + 1) * 1024)
                if MM_F32R:
                    # x feeds f32r matmuls; DMA can't round, so stage then
                    # round-copy on the otherwise idle GPSIMD
                    xstg = xsp.tile([C, 1024], F32, tag="xs")
                    nc.sync.dma_start(xstg[:], xi[:, sl])
                    nc.gpsimd.tensor_copy(x_t[:, sl].bitcast(F32R), xstg[:])
                else:
                    nc.sync.dma_start(x_t[:, sl], xi[:, sl])
            x3 = x_t[:].rearrange("c (h w) -> c h w", h=H)

            # BN+GELU into zero-padded even/odd column buffers:
            #   E[3+r, 1+j] = g[r, 2j]   (width 68, cols 0 and 65.. are pad)
            #   O[3+r, 2+j] = g[r, 2j+1] (cols 0,1 and 66.. are pad)
            # rows 0..2 pad.  eo holds rows [0, EO_R) in f32r for PE; rows
            # [SH_R0, 134) land in bf16 shadows v0 (aligned) / v1 (shifted
            # 1 col left) so the DVE conv runs in 2x mode on any tap shift.
            eo = gp.tile([C, 2, EO_R, 68], F32)
            nc.gpsimd.memset(eo[:, :, 0:3], 0.0)
            nc.gpsimd.memset(eo[:, 0, 3:EO_R, 0:1], 0.0)
            nc.gpsimd.memset(eo[:, 0, 3:EO_R, 65:68], 0.0)
            nc.gpsimd.memset(eo[:, 1, 3:EO_R, 0:2], 0.0)
            nc.gpsimd.memset(eo[:, 1, 3:EO_R, 66:68], 0.0)
            v0 = vshp.tile([C, 2, NSH, 68], BF16, tag="v0")
            v1 = vshp.tile([C, 2, NSH, 68], BF16, tag="v1")
            nc.gpsimd.memset(v0[:, :, NSH - 3:NSH], 0.0)
            nc.gpsimd.memset(v0[:, 0, :, 0:1], 0.0)
            nc.gpsimd.memset(v0[:, 0, :, 65:68], 0.0)
            nc.gpsimd.memset(v0[:, 1, :, 0:2], 0.0)
            nc.gpsimd.memset(v0[:, 1, :, 66:68], 0.0)

            def gelu_seg(dst0, dst1, r0, r1):
                nc.scalar.activation(dst0, x3[:, r0:r1, 0::2], actf.Gelu,
                                     bias=bnb[:, 0:1], scale=bns[:, 0:1])
                nc.scalar.activation(dst1, x3[:, r0:r1, 1::2], actf.Gelu,
                                     bias=bnb[:, 0:1], scale=bns[:, 0:1])

            EOG = EO_R - 3           # g-rows written to eo: [0, EOG)
            for r0, r1 in [(0, 32), (32, 64), (64, EOG)]:
                gelu_seg(_rnd(eo[:, 0, 3 + r0:3 + r1, 1:65], CONV_F32R),
                         _rnd(eo[:, 1, 3 + r0:3 + r1, 2:66], CONV_F32R),
                         r0, r1)
            SHG = SH_R0 - 3          # first g-row in the shadow
            for r0, r1 in [(SHG, 96), (96, 128)]:
                gelu_seg(v0[:, 0, r0 - SHG:r1 - SHG, 1:65],
                         v0[:, 1, r0 - SHG:r1 - SHG, 2:66],
                         r0, r1)
            nc.gpsimd.tensor_copy(v1[:, :, :, 0:67], v0[:, :, :, 1:68])
            nc.gpsimd.memset(v1[:, :, :, 67:68], 0.0)

            oi = out_d[img].rearrange("c h w -> c (h w)")
            if KSTAGE == 1:
                nc.sync.dma_start(
                    oi, eo[:, 0, 3:67, 1:65].rearrange("c h w -> c (h w)"))
                continue

            t0 = t0p.tile([C, L], F32)

            def g_ap(kh, kw, a, b):
                # full-rect tap read for h2 in [a,b), all w2: row 2*h2+kh,
                # col (pad+u)+w2 in the parity buffer
                e = kw - 3
                par, u = (0, e // 2) if e % 2 == 0 else (1, (e - 1) // 2)
                off = (1 + u) if par == 0 else (2 + u)
                if b <= PE_H2:
                    return eo[:, par, kh + 2 * a:kh + 2 * b:2, off:off + 64]
                assert a >= PE_H2
                r0, r1 = kh + 2 * a - SH_R0, kh + 2 * b - SH_R0
                if off % 2 == 0:   # 4B-aligned in v0
                    return v0[:, par, r0:r1:2, off:off + 64]
                return v1[:, par, r0:r1:2, off - 1:off + 63]

            # --- conv: PE part (h2 rows [0, PE_H2)) ---
            blocks = [taps[i:i + TAP_BLOCK] for i in range(0, 49, TAP_BLOCK)]
            for bi, blk in enumerate(blocks):
                dts = []
                for (kh, kw) in blk:
                    dt_ = diagp.tile([C, C], F32, tag="d")
                    nc.vector.tensor_scalar_mul(
                        _rnd(dt_[:], CONV_F32R), ident[:],
                        dww[:, kh * 7 + kw:kh * 7 + kw + 1])
                    dts.append(dt_)
                for a0 in range(0, PE_H2, 8):
                    b0 = min(a0 + 8, PE_H2)
                    w = (b0 - a0) * 64
                    ps = pp512.tile([C, 512], F32)
                    for i, (dt_, (kh, kw)) in enumerate(zip(dts, blk)):
                        nc.tensor.matmul(
                            ps[:, 0:w], _mmdt(dt_[:], CONV_F32R),
                            _mmdt(g_ap(kh, kw, a0, b0), CONV_F32R),
                            start=(i == 0), stop=(i == len(blk) - 1))
                    dst = _rnd(t0[:, a0 * 64:b0 * 64], MM_F32R)
                    if bi == 0:
                        nc.scalar.activation(dst, ps[:, 0:w], actf.Identity,
                                             bias=dwb[:, 0:1])
                    else:
                        nc.vector.tensor_tensor(dst, ps[:, 0:w], dst, alu.add)

            if KSTAGE == 2:
                nc.sync.dma_start(oi, t0[:])
                continue

            # --- conv: DVE part (rows [PE_H2, 64), bf16 2x mode) ---
            nr = 64 - PE_H2
            acc = accp.tile([C, nr, 64], BF16)
            nc.vector.memset(acc[:], 0.0)
            for (kh, kw) in taps:
                nc.vector.scalar_tensor_tensor(
                    acc[:], g_ap(kh, kw, PE_H2, 64),
                    dww[:, kh * 7 + kw:kh * 7 + kw + 1], acc[:],
                    alu.mult, alu.add)
            # merge + dw_b for the DVE range (on ACT: copy with bias)
            nc.scalar.activation(_rnd(t0[:, PE_H2 * 64:], MM_F32R),
                                 acc[:].rearrange("c h w -> c (h w)"),
                                 actf.Identity, bias=dwb[:, 0:1])

            if KSTAGE == 3:
                nc.sync.dma_start(oi, t0[:])
                continue

            # token access patterns (m=0 guide, m>=1 raw-x 2x2 windows)
            def tok_ap(m, c0, c1):
                if m == 0:
                    return t0[:, c0:c1]
                p, q = (m - 1) // 2, (m - 1) % 2
                assert c0 % 64 == 0 and c1 % 64 == 0
                return x3[:, p::2, q::2][:, c0 // 64:c1 // 64, :]

            # --- q0 (evicted to SBUF as bf16 so the dots run in 2x) ---
            q0 = q0p.tile([C, L], BF16)
            q0sums = vecp.tile([C, 4], F32, tag="q0s")
            for ch in range(4):
                ps = ppk.tile([C, 1024], F32, tag="kq")
                for j in range(2):
                    c0 = ch * 1024 + j * 512
                    nc.tensor.matmul(ps[:, j * 512:(j + 1) * 512],
                                     _mmdt(wqT[:], MM_F32R),
                                     _mmdt(t0[:, c0:c0 + 512], MM_F32R),
                                     start=True, stop=True)
                nc.scalar.activation(q0[:, ch * 1024:(ch + 1) * 1024], ps[:],
                                     actf.Identity, bias=bq[:, 0:1],
                                     accum_out=q0sums[:, ch:ch + 1])

            if KSTAGE == 4:
                nc.sync.dma_start(oi, q0[:])
                continue

            # --- k_m + dots (k -> SBUF bf16 via ACT; dot on DVE in 2x) ---
            dots = vecp.tile([C, 20], F32, tag="dots")
            MRANGE = int(os.environ.get("KMRANGE", "5"))
            for m in range(MRANGE):
                for hf in range(4):
                    kp = ppk.tile([C, 1024], F32, tag="kq")
                    for j in range(2):
                        c0 = hf * 1024 + j * 512
                        nc.tensor.matmul(kp[:, j * 512:(j + 1) * 512],
                                         _mmdt(wkT[:], MM_F32R),
                                         _mmdt(tok_ap(m, c0, c0 + 512), MM_F32R),
                                         start=True, stop=True)
                    ksb = ksbp.tile([C, 1024], BF16, tag="k")
                    nc.scalar.copy(ksb[:], kp[:])
                    scr = scrp.tile([C, 1024], BF16, tag="s")
                    nc.vector.scalar_tensor_tensor(
                        scr[:], q0[:, hf * 1024:(hf + 1) * 1024], 1.0, ksb[:],
                        alu.mult, alu.mult,
                        accum_out=dots[:, m * 4 + hf:m * 4 + hf + 1])

            if KSTAGE == 45:
                nc.sync.dma_start(oi[:, 0:20], dots[:])
                nc.sync.dma_start(oi[:, 20:], q0[:, 20:])
                continue

            # --- softmax over 5 logits ---
            s5 = vecp.tile([C, 5], F32, tag="s5")
            nc.vector.tensor_reduce(
                s5[:], dots[:].rearrange("c (m h) -> c m h", m=5),
                mybir.AxisListType.X, alu.add)
            q0s = vecp.tile([C, 1], F32, tag="v")
            nc.vector.tensor_reduce(q0s[:], q0sums[:], mybir.AxisListType.X,
                                    alu.add)
            bkq = vecp.tile([C, 1], F32, tag="v")
            nc.vector.tensor_tensor(bkq[:], bk[:], q0s[:], alu.mult)
            nc.vector.tensor_tensor(s5[:], s5[:],
                                    bkq[:, 0:1].broadcast_to((C, 5)), alu.add)
            if KSTAGE == 46:
                nc.sync.dma_start(oi[:, 0:5], s5[:])
                nc.sync.dma_start(oi[:, 5:], q0[:, 5:])
                continue

            mx = vecp.tile([C, 1], F32, tag="v")
            nc.vector.tensor_reduce(mx[:], s5[:], mybir.AxisListType.X, alu.max)
            nmx = vecp.tile([C, 1], F32, tag="v")
            nc.vector.tensor_scalar_mul(nmx[:], mx[:], -INV_SQRT_C)
            e5 = vecp.tile([C, 5], F32, tag="s5")
            nc.scalar.activation(e5[:], s5[:], actf.Exp, bias=nmx[:, 0:1],
                                 scale=INV_SQRT_C)
            ssum = vecp.tile([C, 1], F32, tag="v")
            nc.vector.tensor_reduce(ssum[:], e5[:], mybir.AxisListType.X,
                                    alu.add)
            sinv = vecp.tile([C, 1], F32, tag="v")
            nc.vector.reciprocal(sinv[:], ssum[:])
            a5 = vecp.tile([C, 5], F32, tag="s5")
            nc.vector.tensor_scalar_mul(a5[:], e5[:], sinv[:, 0:1])

            if KSTAGE == 5:
                nc.sync.dma_start(oi[:, 0:5], a5[:])
                nc.sync.dma_start(oi[:, 5:], q0[:, 5:])
                continue

            # --- fused v+proj: lhsT_m = Wv^T diag(a_m) Pw^T ---
            vts = []
            for m in range(5):
                em = emp.tile([C, C], F32, tag="em")
                nc.vector.tensor_scalar_mul(_rnd(em[:], MM_F32R), pwT[:],
                                             a5[:, m:m + 1])
                vp = pp512.tile([C, 512], F32, tag="w")
                nc.tensor.matmul(vp[:, 0:C], _mmdt(wv_use[:], MM_F32R),
                                 _mmdt(em[:], MM_F32R), start=True, stop=True)
                vt = vtp.tile([C, C], F32, tag="vt")
                nc.scalar.copy(_rnd(vt[:], MM_F32R), vp[:, 0:C])
                vts.append(vt)

            for ch in range(8):
                ps = pp512.tile([C, 512], F32)
                for m in range(5):
                    nc.tensor.matmul(
                        ps[:], _mmdt(vts[m][:], MM_F32R),
                        _mmdt(tok_ap(m, ch * 512, (ch + 1) * 512), MM_F32R),
                        start=(m == 0), stop=(m == 4))
                oc = outp.tile([C, 512], F32, tag="oc")
                nc.scalar.activation(oc[:], ps[:], actf.Identity,
                                     bias=cb[:, 0:1])
                nc.sync.dma_start(oi[:, ch * 512:(ch + 1) * 512], oc[:])
    return nc


_CACHE = {}


def _get_nc():
    if "nc" not in _CACHE:
        tile_utils.max_sbuf_usage = SBUF_CAP
        nc = bacc.Bacc("TRN2", target_bir_lowering=False, debug=False,
                       num_devices=NCORES)
        build(nc)
        nc.compile()
        _CACHE["nc"] = nc
    return _CACHE["nc"]


def kernel(x, bn_gamma, bn_beta, bn_mean, bn_var, dw_w, dw_b, qkv_w, qkv_b,
           proj_w, proj_b):
    nc = _get_nc()
    shared = {
        "bn_gamma": np.asarray(bn_gamma, np.float32).reshape(C, 1),
        "bn_beta": np.asarray(bn_beta, np.float32).reshape(C, 1),
        "bn_mean": np.asarray(bn_mean, np.float32).reshape(C, 1),
        "bn_var": np.asarray(bn_var, np.float32).reshape(C, 1),
        "dw_w": np.asarray(dw_w, np.float32).reshape(C, 49),
        "dw_b": np.asarray(dw_b, np.float32).reshape(C, 1),
        "qkv_w": np.asarray(qkv_w, np.float32).reshape(3 * C, C),
        "qkv_b": np.asarray(qkv_b, np.float32).reshape(3 * C, 1),
        "proj_w": np.asarray(proj_w, np.float32).reshape(C, C),
        "proj_b": np.asarray(proj_b, np.float32).reshape(C, 1),
    }
    xf = np.ascontiguousarray(np.asarray(x, np.float32))
    in_maps = [dict(shared, x=xf[i * BPC:(i + 1) * BPC]) for i in range(NCORES)]
    res = bass_utils.run_bass_kernel_spmd(nc, in_maps,
                                          core_ids=list(range(NCORES)))
    return np.concatenate([r["out"] for r in res.results], axis=0)



# revision 48
# speedup vs baseline: 3.0542x; 3.0542x over previous
"""Trainium2 Bass kernel for nn_LocalFeatureGuided.

Pipeline per image (C=128 on partitions, spatial on free dim):
  BN(eval)+GELU (ACT, fused affine) -> even/odd column split buffers
  depthwise 7x7 s2 conv: 49 taps split by output-column ranges across
    PE (diag-matmul, PSUM accum), DVE (scalar_tensor_tensor FMA), GPSIMD
  tokens: t0=guide, t1..4 = strided views of x (no copies)
  q0 = WqT.T@t0 ; k_m = WkT.T@t_m consumed from PSUM by
    tensor_tensor_reduce dots -> s_m = <q0,k_m>
  softmax over 5 logits per (b,c); v & proj fused:
    out = sum_m (Wv^T diag(a_m) Pw^T)^T @ t_m  (5 accumulating matmuls)
Sharding: data-parallel over batch, 2 images per core, 8 cores.
"""

import os
import numpy as np
from contextlib import ExitStack

import concourse.bass as bass
import concourse.tile as tile
from concourse import bacc, mybir
from concourse import bass_utils
from concourse import tile_utils

alu = mybir.AluOpType
actf = mybir.ActivationFunctionType
F32 = mybir.dt.float32
F32R = mybir.dt.float32r
BF16 = mybir.dt.bfloat16

B, C, H, W = 16, 128, 128, 128
H2, W2 = H // 2, W // 2
L = H2 * W2            # 4096
NCORES = 8
BPC = B // NCORES      # 2 images per core
EPS = 1e-5
INV_SQRT_C = 1.0 / np.sqrt(128.0)

KSTAGE = int(os.environ.get("KSTAGE", "9"))
KREPEAT = int(os.environ.get("KREPEAT", "1"))  # timing: repeat image loop

# ---- tuning knobs ----
PE_H2 = 48             # conv: h2 rows 0..PE_H2 on PE; DVE does the rest
MM_F32R = False        # retired: datapath is bf16 end-to-end
CONV_F32R = False
TAP_BLOCK = 17         # diag weight tiles alive at once
SBUF_CAP = 204 * 1024  # cayman has 208 KiB usable per partition

KH_LO = [2, 1, 1, 0, 0, 0, 0]
KH_HI = [64, 64, 64, 64, 64, 63, 63]


def _mmdt(ap, enable):
    return ap.bitcast(F32R) if enable else ap


_rnd = _mmdt  # producers of matmul operands must round to f32r on write


def tap_geometry(kh, kw):
    """Returns (parity, u, h2 range, w2 range) for tap (kh, kw)."""
    e = kw - 3
    if e % 2 == 0:
        par, u = 0, e // 2          # even: reads E[r, w2+u], u in -1..1
    else:
        par, u = 1, (e - 1) // 2    # odd: reads O[r, w2+u], u in -2..1
    wlo, whi = max(0, -u), min(64, 64 - u)
    return par, u, KH_LO[kh], KH_HI[kh], wlo, whi


def build(nc):
    x_d = nc.dram_tensor("x", (BPC, C, H, W), F32, kind="ExternalInput").ap()
    bng_d = nc.dram_tensor("bn_gamma", (C, 1), F32, kind="ExternalInput").ap()
    bnb_d = nc.dram_tensor("bn_beta", (C, 1), F32, kind="ExternalInput").ap()
    bnm_d = nc.dram_tensor("bn_mean", (C, 1), F32, kind="ExternalInput").ap()
    bnv_d = nc.dram_tensor("bn_var", (C, 1), F32, kind="ExternalInput").ap()
    dww_d = nc.dram_tensor("dw_w", (C, 49), F32, kind="ExternalInput").ap()
    dwb_d = nc.dram_tensor("dw_b", (C, 1), F32, kind="ExternalInput").ap()
    qkvw_d = nc.dram_tensor("qkv_w", (3 * C, C), F32, kind="ExternalInput").ap()
    qkvb_d = nc.dram_tensor("qkv_b", (3 * C, 1), F32, kind="ExternalInput").ap()
    pw_d = nc.dram_tensor("proj_w", (C, C), F32, kind="ExternalInput").ap()
    pb_d = nc.dram_tensor("proj_b", (C, 1), F32, kind="ExternalInput").ap()
    out_d = nc.dram_tensor("out", (BPC, C, H2, W2), F32, kind="ExternalOutput").ap()

    with tile.TileContext(nc) as tc, ExitStack() as ctx:
        tp = lambda name, bufs, **kw: ctx.enter_context(
            tc.tile_pool(name=name, bufs=bufs, **kw))

        wpool = tp("weights", 1)       # persistent small weights
        diagp = tp("diag", 49)  # all conv diag weight tiles
        xp = tp("x", 2)
        gp = tp("gelu", 2)
        t0p = tp("t0", 2)
        q0p = tp("q0", 2)
        accp = tp("acc", 1)
        outp = tp("outc", 3)
        vecp = tp("vec", 24)
        emp = tp("em", 2)
        scrp = tp("scr", 1)
        vtp = tp("vt", 5)
        pp512 = tp("pp512", 4, space="PSUM")
        ppk = tp("ppk", 2, space="PSUM")   # [128,1024] = 2 banks each

        # ---------- phase 0: weights & per-channel vectors ----------
        def vec_load(src_ap):
            t = vecp.tile([C, 1], F32, tag="v")
            nc.sync.dma_start(t[:], src_ap)
            return t

        gam = vec_load(bng_d)
        bet = vec_load(bnb_d)
        mea = vec_load(bnm_d)
        var = vec_load(bnv_d)
        dwb = vec_load(dwb_d)
        bq = vec_load(qkvb_d[0:C])
        bk = vec_load(qkvb_d[C:2 * C])
        bv = vec_load(qkvb_d[2 * C:3 * C])
        pb = vec_load(pb_d)

        dww = wpool.tile([C, 49], F32)
        nc.sync.dma_start(dww[:], dww_d)

        wq_raw = wpool.tile([C, C], F32)
        nc.sync.dma_start(wq_raw[:], qkvw_d[0:C])
        wk_raw = wpool.tile([C, C], F32)
        nc.sync.dma_start(wk_raw[:], qkvw_d[C:2 * C])
        wv_raw = wpool.tile([C, C], F32)          # used directly as lhsT
        nc.sync.dma_start(wv_raw[:], qkvw_d[2 * C:3 * C])
        pw_raw = wpool.tile([C, C], F32)
        nc.sync.dma_start(pw_raw[:], pw_d)

        # identity for transposes / diag builds
        ones = wpool.tile([C, C], F32)
        nc.gpsimd.memset(ones[:], 1.0)
        ident = wpool.tile([C, C], F32)
        nc.gpsimd.affine_select(ident[:], ones[:], [[-1, C]], alu.is_equal,
                                0.0, base=0, channel_multiplier=1)

        def pe_transpose(src, rnd_out=False):
            ps = ppk.tile([C, 1024], F32, tag="kq")
            nc.tensor.matmul(ps[:, 0:C], src[:], ident[:], start=True,
                             stop=True, is_transpose=True)
            dst = wpool.tile([C, C], BF16 if rnd_out else F32,
                             tag=f"T{src.name}")
            nc.scalar.copy(dst[:], ps[:, 0:C])
            return dst

        wqT = pe_transpose(wq_raw, rnd_out=True)      # [j, cq] bf16
        wkT = pe_transpose(wk_raw, rnd_out=True)      # [j, ck] bf16
        pwT = pe_transpose(pw_raw)                    # [o, d]
        wv_use = wpool.tile([C, C], BF16, tag="wvr")
        nc.scalar.copy(wv_use[:], wv_raw[:])

        # BN affine: scale = gamma*rsqrt(var+eps); bias = beta - mean*scale
        epst = vecp.tile([C, 1], F32, tag="v")
        nc.gpsimd.memset(epst[:], EPS)
        std = vecp.tile([C, 1], F32, tag="v")
        nc.scalar.activation(std[:], var[:], actf.Sqrt, bias=epst[:, 0:1],
                             scale=1.0)
        istd = vecp.tile([C, 1], F32, tag="v")
        nc.vector.reciprocal(istd[:], std[:])
        bns = vecp.tile([C, 1], F32, tag="v")
        nc.vector.tensor_tensor(bns[:], gam[:], istd[:], alu.mult)
        bnsn = vecp.tile([C, 1], F32, tag="v")
        nc.vector.tensor_scalar_mul(bnsn[:], bns[:], -1.0)
        bnb = vecp.tile([C, 1], F32, tag="v")
        nc.vector.scalar_tensor_tensor(bnb[:], mea[:], bnsn[:], bet[:],
                                       alu.mult, alu.add)

        # cb = Pw @ bv + pb   (softmax weights sum to 1 -> v-bias folds out)
        cps = ppk.tile([C, 1024], F32, tag="kq")
        nc.tensor.matmul(cps[:, 0:1], pwT[:], bv[:], start=True, stop=True)
        cb = vecp.tile([C, 1], F32, tag="v")
        nc.scalar.activation(cb[:], cps[:, 0:1], actf.Identity,
                             bias=pb[:, 0:1])

        taps = [(kh, kw) for kh in range(7) for kw in range(7)]
        alldts = {}
        for (kh, kw) in taps:
            dt_ = diagp.tile([C, C], BF16, tag="d")
            nc.vector.tensor_scalar_mul(
                dt_[:], ident[:], dww[:, kh * 7 + kw:kh * 7 + kw + 1])
            alldts[(kh, kw)] = dt_

        # ---------- per image ----------
        for img in [i % BPC for i in range(BPC * KREPEAT)]:
            x_t = xp.tile([C, H * W], BF16)
            xi = x_d[img].rearrange("c h w -> c (h w)")
            for s in range(16):  # parallel DMA queues
                sl = slice(s * 1024, (s + 1) * 1024)
                # SWDGE (gpsimd-initiated) DMA casts f32 -> bf16 inline
                nc.gpsimd.dma_start(x_t[:, sl], xi[:, sl])
            x3 = x_t[:].rearrange("c (h w) -> c h w", h=H)

            # BN+GELU into zero-padded even/odd column buffers:
            #   E[3+r, 1+j] = g[r, 2j]   (width 68, cols 0 and 65.. are pad)
            #   O[3+r, 2+j] = g[r, 2j+1] (cols 0,1 and 66.. are pad)
            # rows 0..2 and 131..133 are pad.
            eo = gp.tile([C, 2, 134, 68], BF16)
            nc.gpsimd.memset(eo[:, :, 0:3], 0.0)
            nc.gpsimd.memset(eo[:, :, 131:134], 0.0)
            nc.gpsimd.memset(eo[:, 0, 3:131, 0:1], 0.0)
            nc.gpsimd.memset(eo[:, 0, 3:131, 65:68], 0.0)
            nc.gpsimd.memset(eo[:, 1, 3:131, 0:2], 0.0)
            nc.gpsimd.memset(eo[:, 1, 3:131, 66:68], 0.0)
            for bnd in range(4):  # row bands so conv can start early
                r0, r1 = 32 * bnd, 32 * (bnd + 1)
                nc.scalar.activation(
                    _rnd(eo[:, 0, 3 + r0:3 + r1, 1:65], CONV_F32R),
                    x3[:, r0:r1, 0::2], actf.Gelu,
                    bias=bnb[:, 0:1], scale=bns[:, 0:1])
                nc.scalar.activation(
                    _rnd(eo[:, 1, 3 + r0:3 + r1, 2:66], CONV_F32R),
                    x3[:, r0:r1, 1::2], actf.Gelu,
                    bias=bnb[:, 0:1], scale=bns[:, 0:1])

            oi = out_d[img].rearrange("c h w -> c (h w)")
            if KSTAGE == 15:   # timing: BN+GELU only
                for si, src in enumerate([eo[:, 0, 0:4, 0:4],
                                          eo[:, 1, 0:4, 0:4]]):
                    nc.gpsimd.dma_start(
                        oi[:, 16 * si:16 * (si + 1)].rearrange(
                            "c (h w) -> c h w", h=4), src)
                nc.gpsimd.dma_start(oi[:, 64:L], x_t[:, 64:L])
                continue
            if KSTAGE == 1:
                nc.sync.dma_start(
                    oi, eo[:, 0, 3:67, 1:65].rearrange("c h w -> c (h w)"))
                continue

            t0 = t0p.tile([C, L], BF16)

            def g_ap(kh, kw, a, b):
                # full-rect tap read for h2 in [a,b), all w2: row 2*h2+kh,
                # col (pad+u)+w2 in the parity buffer
                e = kw - 3
                par, u = (0, e // 2) if e % 2 == 0 else (1, (e - 1) // 2)
                off = (1 + u) if par == 0 else (2 + u)
                return eo[:, par, kh + 2 * a:kh + 2 * b:2, off:off + 64]

            # --- conv: PE part (h2 rows [0, PE_H2)) ---
            blocks = [taps[i:i + TAP_BLOCK] for i in range(0, 49, TAP_BLOCK)]
            for bi, blk in enumerate(blocks):
                dts = [alldts[t_] for t_ in blk]
                for a0 in range(0, PE_H2, 8):
                    b0 = min(a0 + 8, PE_H2)
                    w = (b0 - a0) * 64
                    ps = pp512.tile([C, 512], F32)
                    for i, (dt_, (kh, kw)) in enumerate(zip(dts, blk)):
                        nc.tensor.matmul(
                            ps[:, 0:w], _mmdt(dt_[:], CONV_F32R),
                            _mmdt(g_ap(kh, kw, a0, b0), CONV_F32R),
                            start=(i == 0), stop=(i == len(blk) - 1))
                    dst = _rnd(t0[:, a0 * 64:b0 * 64], MM_F32R)
                    if bi == 0:
                        nc.scalar.activation(dst, ps[:, 0:w], actf.Identity,
                                             bias=dwb[:, 0:1])
                    else:
                        nc.vector.tensor_tensor(dst, ps[:, 0:w], dst, alu.add)

            if KSTAGE == 2:
                nc.sync.dma_start(oi, t0[:])
                continue

            # --- conv: DVE part (rows [PE_H2, 64)) ---
            nr = 64 - PE_H2
            acc = accp.tile([C, nr, 64], F32)
            nc.vector.memset(acc[:], 0.0)
            for (kh, kw) in taps:
                nc.vector.scalar_tensor_tensor(
                    acc[:], g_ap(kh, kw, PE_H2, 64),
                    dww[:, kh * 7 + kw:kh * 7 + kw + 1], acc[:],
                    alu.mult, alu.add)
            # merge + dw_b for the DVE range (on ACT: copy with bias)
            nc.scalar.activation(_rnd(t0[:, PE_H2 * 64:], MM_F32R),
                                 acc[:].rearrange("c h w -> c (h w)"),
                                 actf.Identity, bias=dwb[:, 0:1])

            if KSTAGE == 3:
                nc.sync.dma_start(oi, t0[:])
                continue

            # token access patterns (m=0 guide, m>=1 raw-x 2x2 windows)
            def tok_ap(m, c0, c1):
                if m == 0:
                    return t0[:, c0:c1]
                p, q = (m - 1) // 2, (m - 1) % 2
                assert c0 % 64 == 0 and c1 % 64 == 0
                return x3[:, p::2, q::2][:, c0 // 64:c1 // 64, :]

            # --- q0 ---
            q0 = q0p.tile([C, L], BF16)
            q0sums = vecp.tile([C, 4], F32, tag="q0s")
            for ch in range(4):
                ps = ppk.tile([C, 1024], F32, tag="kq")
                for j in range(2):
                    c0 = ch * 1024 + j * 512
                    nc.tensor.matmul(ps[:, j * 512:(j + 1) * 512],
                                     _mmdt(wqT[:], MM_F32R),
                                     _mmdt(t0[:, c0:c0 + 512], MM_F32R),
                                     start=True, stop=True)
                nc.scalar.activation(q0[:, ch * 1024:(ch + 1) * 1024], ps[:],
                                     actf.Identity, bias=bq[:, 0:1],
                                     accum_out=q0sums[:, ch:ch + 1])

            if KSTAGE == 4:
                nc.sync.dma_start(oi, q0[:])
                continue

            # --- k_m + dots (k -> SBUF bf16 via ACT; dot on DVE in 2x) ---
            dots = vecp.tile([C, 20], F32, tag="dots")
            MRANGE = int(os.environ.get("KMRANGE", "5"))
            for m in range(MRANGE):
                for hf in range(4):
                    kp = ppk.tile([C, 1024], F32, tag="kq")
                    for j in range(2):
                        c0 = hf * 1024 + j * 512
                        nc.tensor.matmul(kp[:, j * 512:(j + 1) * 512],
                                         _mmdt(wkT[:], MM_F32R),
                                         _mmdt(tok_ap(m, c0, c0 + 512), MM_F32R),
                                         start=True, stop=True)
                    scr = scrp.tile([C, 1024], F32, tag="s")
                    nc.vector.scalar_tensor_tensor(
                        scr[:], q0[:, hf * 1024:(hf + 1) * 1024], 1.0, kp[:],
                        alu.mult, alu.mult,
                        accum_out=dots[:, m * 4 + hf:m * 4 + hf + 1])

            if KSTAGE == 45:
                nc.sync.dma_start(oi[:, 0:20], dots[:])
                nc.sync.dma_start(oi[:, 20:], q0[:, 20:])
                continue

            # --- softmax over 5 logits ---
            s5 = vecp.tile([C, 5], F32, tag="s5")
            nc.vector.tensor_reduce(
                s5[:], dots[:].rearrange("c (m h) -> c m h", m=5),
                mybir.AxisListType.X, alu.add)
            q0s = vecp.tile([C, 1], F32, tag="v")
            nc.vector.tensor_reduce(q0s[:], q0sums[:], mybir.AxisListType.X,
                                    alu.add)
            bkq = vecp.tile([C, 1], F32, tag="v")
            nc.vector.tensor_tensor(bkq[:], bk[:], q0s[:], alu.mult)
            nc.vector.tensor_tensor(s5[:], s5[:],
                                    bkq[:, 0:1].broadcast_to((C, 5)), alu.add)
            if KSTAGE == 46:
                nc.sync.dma_start(oi[:, 0:5], s5[:])
                nc.sync.dma_start(oi[:, 5:], q0[:, 5:])
                continue

            mx = vecp.tile([C, 1], F32, tag="v")
            nc.vector.tensor_reduce(mx[:], s5[:], mybir.AxisListType.X, alu.max)
            nmx = vecp.tile([C, 1], F32, tag="v")
            nc.vector.tensor_scalar_mul(nmx[:], mx[:], -INV_SQRT_C)
            e5 = vecp.tile([C, 5], F32, tag="s5")
            nc.scalar.activation(e5[:], s5[:], actf.Exp, bias=nmx[:, 0:1],
                                 scale=INV_SQRT_C)
            ssum = vecp.tile([C, 1], F32, tag="v")
            nc.vector.tensor_reduce(ssum[:], e5[:], mybir.AxisListType.X,
                                    alu.add)
            sinv = vecp.tile([C, 1], F32, tag="v")
            nc.vector.reciprocal(sinv[:], ssum[:])
            a5 = vecp.tile([C, 5], F32, tag="s5")
            nc.vector.tensor_scalar_mul(a5[:], e5[:], sinv[:, 0:1])

            if KSTAGE == 5:
                nc.sync.dma_start(oi[:, 0:5], a5[:])
                nc.sync.dma_start(oi[:, 5:], q0[:, 5:])
                continue

            # --- fused v+proj: lhsT_m = Wv^T diag(a_m) Pw^T ---
            vts = []
            for m in range(5):
                em = emp.tile([C, C], BF16, tag="em")
                nc.vector.tensor_scalar_mul(_rnd(em[:], MM_F32R), pwT[:],
                                             a5[:, m:m + 1])
                vp = ppk.tile([C, 1024], F32, tag="kq")
                nc.tensor.matmul(vp[:, 0:C], _mmdt(wv_use[:], MM_F32R),
                                 _mmdt(em[:], MM_F32R), start=True, stop=True)
                vt = vtp.tile([C, C], BF16, tag="vt")
                nc.scalar.copy(_rnd(vt[:], MM_F32R), vp[:, 0:C])
                vts.append(vt)

            for ch in range(8):
                ps = pp512.tile([C, 512], F32)
                for m in range(5):
                    nc.tensor.matmul(
                        ps[:], _mmdt(vts[m][:], MM_F32R),
                        _mmdt(tok_ap(m, ch * 512, (ch + 1) * 512), MM_F32R),
                        start=(m == 0), stop=(m == 4))
                oc = outp.tile([C, 512], F32, tag="oc")
                nc.scalar.activation(oc[:], ps[:], actf.Identity,
                                     bias=cb[:, 0:1])
                nc.sync.dma_start(oi[:, ch * 512:(ch + 1) * 512], oc[:])
    return nc


_CACHE = {}


def _get_nc():
    if "nc" not in _CACHE:
        tile_utils.max_sbuf_usage = SBUF_CAP
        nc = bacc.Bacc("TRN2", target_bir_lowering=False, debug=False,
                       num_devices=NCORES)
        build(nc)
        nc.compile()
        _CACHE["nc"] = nc
    return _CACHE["nc"]


def kernel(x, bn_gamma, bn_beta, bn_mean, bn_var, dw_w, dw_b, qkv_w, qkv_b,
           proj_w, proj_b):
    nc = _get_nc()
    shared = {
        "bn_gamma": np.asarray(bn_gamma, np.float32).reshape(C, 1),
        "bn_beta": np.asarray(bn_beta, np.float32).reshape(C, 1),
        "bn_mean": np.asarray(bn_mean, np.float32).reshape(C, 1),
        "bn_var": np.asarray(bn_var, np.float32).reshape(C, 1),
        "dw_w": np.asarray(dw_w, np.float32).reshape(C, 49),
        "dw_b": np.asarray(dw_b, np.float32).reshape(C, 1),
        "qkv_w": np.asarray(qkv_w, np.float32).reshape(3 * C, C),
        "qkv_b": np.asarray(qkv_b, np.float32).reshape(3 * C, 1),
        "proj_w": np.asarray(proj_w, np.float32).reshape(C, C),
        "proj_b": np.asarray(proj_b, np.float32).reshape(C, 1),
    }
    xf = np.ascontiguousarray(np.asarray(x, np.float32))
    in_maps = [dict(shared, x=xf[i * BPC:(i + 1) * BPC]) for i in range(NCORES)]
    res = bass_utils.run_bass_kernel_spmd(nc, in_maps,
                                          core_ids=list(range(NCORES)))
    return np.concatenate([r["out"] for r in res.results], axis=0)

